# revision 81
# baseline (speedup 1.0000x reference)
"""Trainium2 Bass kernel for nn_DepthWiseConv_AConnect (depthwise 3x3 conv with
per-pool multiplicative weight/bias noise, followed by 8-bit LQuant).

Strategy (8 NeuronCores, data-parallel over the pool axis; core p handles
pool group p: 8 images, Werr[p], Berr[p]). X ships fp16 (exact products in
the fp32 PSUM accumulator), outputs ship int8. The 9 conv taps are split
across three engines so all of them stay ~saturated:
  - TensorEngine: taps 0..5 as accumulating diagonal matmuls per
    9-output-row PSUM bank (weights pre-scaled by 127).
  - ScalarE: taps 6, 7 via activation with per-partition scale (+bias),
    fp16 results into SBUF. On alternate units (h==1) tap 8 also runs
    here so the DVE's per-unit time stays below the PE period.
  - DVE: tap 8 fused with the tap-6 merge (scalar_tensor_tensor) on h==0
    units / two cheap 2x-mode tensor_tensor merges on h==1 units, then a
    single fused custom-DVE op for the whole LQuant: adds psum+acc,
    rounds to the integer grid via the 1.5*2^23 magic constant (RNE),
    clips to [-127, 127], emits int8.
  - ScalarE->PSUM seeding was tried and is NOT safe: PE start=False
    accumulation races engine PSUM writes nondeterministically.
  - Host divides by 127 and transposes back to NHWC.
HW exec time ~144 us vs 723 us for the original 3-op quant chain version
(GpSimd tensor_scalar was the 685-us bottleneck; GpSimd runs ~16x slower
than ScalarE for bulk elementwise work - avoid it entirely).
"""
import re
import sys

import numpy as np

try:
    import concourse.bacc as bacc_mod
except ImportError:
    sys.path.insert(0, "/opt/trn_rl_repo")
    import concourse.bacc as bacc_mod

import concourse.mybir as mybir
from concourse.tile import TileContext
from concourse.bass_utils import run_bass_kernel_spmd
from contextlib import ExitStack

POOL = 8
NB = 8            # images per pool group (64 / 8)
H = W = 56
HO = WO = 54
C = 256
NCH = 2           # channel chunks of 128
NPIX = H * W      # 3136
NOUT = HO * WO    # 2916
BANKN = 486       # output pixels per psum bank (9 rows x 54)
HALFN = 3 * BANKN  # 1458 output pixels per half (27 rows)
MAGIC = 12582912.0  # 1.5 * 2^23
S = 127.0

f32 = mybir.dt.float32
f16 = mybir.dt.float16
i8 = mybir.dt.int8

_cached = {}


def _register_quant_ops():
    """Register the fused LQuant custom-DVE ops (idempotent)."""
    from concourse import dve_ops
    from concourse.dve_spec import Spec, Src0, Src1, C0, C1, maxx, minn
    from concourse.bass import dve_ver_for

    def reg(name, spec):
        for op in dve_ops.OPS:
            if op.name == name:
                return op
        op = dve_ops.DveOp(name, spec, subdim=False, uops_sha={})
        dve_ops.OPS.append(op)
        dve_ops.CUSTOM_DVE_SPECS[name] = spec
        dve_ops._SUB_OPCODE_FOR_NAME[name] = (
            dve_ops._CUSTOM_DVE_ROW_BASE + len(dve_ops.OPS) - 1
        )
        ver = dve_ver_for("TRN2")
        try:
            op.compile(ver)
        except ValueError as e:  # harvest the computed sha from the message
            m = re.findall(r'="([0-9a-f]+)"', str(e))
            assert m, f"could not parse uops sha from: {e}"
            op.uops_sha[ver] = m[-1]
            dve_ops._COMPILE_CACHE.pop((name, ver), None)
            op.compile(ver)
        return op

    def _ref_q(in0, in1, s0, s1, imm2):
        t = in0.astype(np.float32) + np.float32(s0)
        t = np.minimum(np.maximum(t, np.float32(s1)), np.float32(imm2))
        return t - np.float32(s0)

    def _ref_qa(in0, in1, s0, s1, imm2):
        t = in0.astype(np.float32) + in1.astype(np.float32)
        t = (t + np.float32(s0)) - np.float32(s0)
        return np.minimum(np.maximum(t, np.float32(-s1)), np.float32(s1))

    from concourse.dve_spec import Zero, C2
    q = reg("LQUANT_MAGIC_ANT", Spec(
        body=minn(maxx(Src0 + C0, C1), C2) - C0,
        reference=_ref_q))
    qa = reg("LQUANT_MAGIC_ACC_ANT", Spec(
        body=minn(maxx(((Src0 + Src1) + C0) - C0, Zero - C1), C1),
        reference=_ref_qa))
    return q, qa


def _build():
    quant_op, quant_acc_op = _register_quant_ops()

    nc = bacc_mod.Bacc()
    # X pre-split into overlapping row-halves (0:29, 27:56) per chunk
    xt = nc.dram_tensor("xt", [NB, NCH, 2, 128, 29 * W], f16,
                        kind="ExternalInput")
    # one fused consts blob, partition-major: 7 diag weight slots (fp16),
    # the per-channel f32 scalars [tap6 w, tap7 w, tap8 w, bias] bitcast
    # into the fp16 stream, then the FIRST unit's half-image — a single
    # DMA covers the whole cold start (one DGE slot, one sem round)
    NWS = NCH * 7 * 128
    NSC = NCH * 10
    wcb = nc.dram_tensor("wcb", [128, NWS + NSC + 29 * W], f16,
                         kind="ExternalInput")
    out = nc.dram_tensor("out", [NB, NCH, 128, NOUT], i8, kind="ExternalOutput")

    with TileContext(nc) as tc, ExitStack() as ctx:
        consts = ctx.enter_context(tc.tile_pool(name="consts", bufs=1))
        xpool = ctx.enter_context(tc.tile_pool(name="xpool", bufs=4))
        opool = ctx.enter_context(tc.tile_pool(name="opool", bufs=4))
        vpool = ctx.enter_context(tc.tile_pool(name="vpool", bufs=4))
        pspool = ctx.enter_context(tc.tile_pool(name="pspool", bufs=2, space="PSUM"))

        blob = consts.tile([128, NWS + NSC + 29 * W], f16)
        nc.sync.dma_start(out=blob, in_=wcb[:, :])
        ws = blob[:, 0:NWS].rearrange("p (q t m) -> p q t m", q=NCH, t=7)
        sc = blob[:, NWS:NWS + NSC].bitcast(f32).rearrange(
            "p (q s) -> p q s", q=NCH)
        xs_first = blob[:, NWS + NSC:]

        for n in range(NB):
            for q in range(NCH):
                w5 = sc[:, q, 0:1]
                w6 = sc[:, q, 1:2]
                w7 = sc[:, q, 2:3]
                w8 = sc[:, q, 3:4]
                bv = sc[:, q, 4:5]
                for h in range(2):
                    if n == 0 and q == 0 and h == 0:
                        xr = xs_first.rearrange("p (h w) -> p h w", w=W)
                    else:
                        xs = xpool.tile([128, 29 * W], f16, tag=f"xs{h}")
                        nc.sync.dma_start(out=xs, in_=xt[n, q, h])
                        xr = xs.rearrange("p (h w) -> p h w", w=W)
                    # taps 6 (2,0), 7 (2,1) on ScalarE; tap 8 (2,2) + merge
                    # + bias on the DVE; all in fp16 SBUF.
                    r0 = 2
                    t6a = vpool.tile([128, 27, WO], f16, tag="t6a")
                    nc.scalar.activation(
                        out=t6a, in_=xr[:, r0:r0 + 27, 0:WO],
                        func=mybir.ActivationFunctionType.Identity,
                        bias=bv, scale=w6)
                    t7a = vpool.tile([128, 27, WO], f16, tag="t7a")
                    nc.scalar.activation(
                        out=t7a, in_=xr[:, r0:r0 + 27, 1:1 + WO],
                        func=mybir.ActivationFunctionType.Identity,
                        bias=0.0, scale=w7)
                    t8_on_pe = False
                    pe5 = (n == NB - 1 and q == NCH - 1)
                    accp = vpool.tile([128, 27, WO], f16, tag="accp")
                    acc = vpool.tile([128, 27, WO], f16, tag="acc")
                    dst0 = accp if pe5 else acc
                    if t8_on_pe:
                        nc.vector.tensor_tensor(out=dst0, in0=t6a, in1=t7a,
                                                op=mybir.AluOpType.add)
                    elif h == 1:
                        # alternate units: tap 8 also on ScalarE; the DVE
                        # does two cheap 2x-mode merges instead of the 1x
                        # STT, keeping the DVE period under the PE's.
                        t8a = vpool.tile([128, 27, WO], f16, tag="t8a")
                        nc.scalar.activation(
                            out=t8a, in_=xr[:, r0:r0 + 27, 2:2 + WO],
                            func=mybir.ActivationFunctionType.Identity,
                            bias=0.0, scale=w8)
                        m1 = vpool.tile([128, 27, WO], f16, tag="m1")
                        nc.vector.tensor_tensor(out=m1, in0=t8a, in1=t6a,
                                                op=mybir.AluOpType.add)
                        nc.vector.tensor_tensor(out=dst0, in0=m1, in1=t7a,
                                                op=mybir.AluOpType.add)
                    else:
                        m1 = vpool.tile([128, 27, WO], f16, tag="m1")
                        nc.vector.scalar_tensor_tensor(
                            out=m1, in0=xr[:, r0:r0 + 27, 2:2 + WO],
                            scalar=w8, in1=t6a,
                            op0=mybir.AluOpType.mult, op1=mybir.AluOpType.add)
                        nc.vector.tensor_tensor(out=dst0, in0=m1, in1=t7a,
                                                op=mybir.AluOpType.add)
                    if pe5:
                        # drain-side: tap 5 (1,2) via ScalarE + one merge
                        t5a = vpool.tile([128, 27, WO], f16, tag="t5a")
                        nc.scalar.activation(
                            out=t5a, in_=xr[:, 1:28, 2:2 + WO],
                            func=mybir.ActivationFunctionType.Identity,
                            bias=0.0, scale=w5)
                        nc.vector.tensor_tensor(out=acc, in0=dst0, in1=t5a,
                                                op=mybir.AluOpType.add)
                    ps = pspool.tile([128, 3, 512], f32, tag="ps")
                    taps = [(0, 0), (0, 1), (0, 2), (1, 0), (1, 1), (1, 2)]
                    if pe5:
                        taps = taps[:-1]
                    if t8_on_pe:
                        taps = taps + [(2, 2)]
                    for b3 in range(3):
                        for t, (i, j) in enumerate(taps):
                            rhs = xr[:, 9 * b3 + i: 9 * b3 + i + 9, j: j + WO]
                            nc.tensor.matmul(ps[:, b3, 0:BANKN],
                                             lhsT=ws[:, q, t, :], rhs=rhs,
                                             start=(t == 0),
                                             stop=(t == len(taps) - 1),
                                             skip_group_check=True)
                    ot = opool.tile([128, 3, BANKN], i8, tag="ot")
                    nc.vector._custom_dve(
                        quant_acc_op, out=ot, in0=ps[:, :, 0:BANKN],
                        in1=acc.rearrange("p (c r) w -> p c (r w)", r=9),
                        s0=MAGIC, s1=S)
                    nc.sync.dma_start(
                        out=out[n, q][:, HALFN * h: HALFN * (h + 1)]
                        .rearrange("p (a b) -> p a b", b=BANKN),
                        in_=ot)

    nc.finalize()
    return nc


def kernel(X, W, bias, Werr, Berr, _trace=False):
    X = np.asarray(X, np.float32)
    W = np.asarray(W, np.float32)
    bias = np.asarray(bias, np.float32)
    Werr = np.asarray(Werr, np.float32)
    Berr = np.asarray(Berr, np.float32)

    if "nc" not in _cached:
        _cached["nc"] = _build()
    nc = _cached["nc"]

    Xh = X.astype(np.float16)  # [64, 56, 56, 256]
    w3 = W[..., 0]             # [3, 3, 256]
    we3 = Werr[..., 0]         # [8, 3, 3, 256]

    in_maps = []
    for p in range(POOL):
        xp = Xh[p * NB:(p + 1) * NB].reshape(NB, NPIX, C)
        xp = np.ascontiguousarray(xp.transpose(0, 2, 1)).reshape(
            (NB, NCH, 128, 56, 56))
        # overlapping row-halves: [0:29] and [27:56]
        xp = np.stack([xp[:, :, :, 0:29, :], xp[:, :, :, 27:56, :]],
                      axis=2).reshape((NB, NCH, 2, 128, 29 * 56))

        w_eff = np.float32(S) * w3 * we3[p]  # [3, 3, 256] fp32
        w_eff16 = w_eff.astype(np.float16)
        wdg = np.zeros((NCH, 7, 128, 128), np.float16)
        for q in range(NCH):
            for t in range(6):
                i, j = divmod(t, 3)
                np.fill_diagonal(wdg[q, t], w_eff16[i, j, 128 * q:128 * (q + 1)])
            np.fill_diagonal(wdg[q, 6], w_eff16[2, 2, 128 * q:128 * (q + 1)])
        wdg = np.ascontiguousarray(wdg.transpose(2, 0, 1, 3))  # [128,NCH,7,128]

        b_eff = (np.float32(S) * bias * Berr[p]).astype(np.float32)
        wsc = np.stack([w_eff[1, 2].astype(np.float32),
                        w_eff[2, 0].astype(np.float32),
                        w_eff[2, 1].astype(np.float32),
                        w_eff[2, 2].astype(np.float32), b_eff],
                       axis=-1).reshape(NCH, 128, 5)
        wsc = np.ascontiguousarray(wsc.transpose(1, 0, 2))  # [128, NCH, 4]
        wcb = np.concatenate(
            [wdg.reshape(128, -1),
             np.ascontiguousarray(wsc).view(np.float16).reshape(128, -1),
             xp[0, 0, 0]],
            axis=1)
        in_maps.append({"xt": xp, "wcb": wcb})

    res = run_bass_kernel_spmd(nc, in_maps, core_ids=list(range(POOL)),
                               trace=_trace)
    if _trace:
        _cached["last_result"] = res

    outs = []
    for p in range(POOL):
        o = res.results[p]["out"].astype(np.float32)  # [NB, NCH, 128, NOUT] int8
        o = o / np.float32(S)
        o = o.reshape(NB, C, HO, WO).transpose(0, 2, 3, 1)  # NHWC
        outs.append(o)
    return np.ascontiguousarray(np.concatenate(outs, axis=0).astype(np.float32))
